# revision 1
# baseline (speedup 1.0000x reference)
"""Trainium2 Bass kernel for MultiHeadLatentAttention (MLA) forward.

Sharding: 8 cores = 2 (batch) x 4 (head-groups of 4 heads).
Each core runs the same SPMD Bass program on its shard:
  P (per s-quarter): fused down-proj (q_down | kv_down | k_rope) + RMSNorm
      + RoPE, then immediately the up-projections (q_up, kv_up k_nope + v)
      on the in-SBUF latents; Q^T / K^T / V spill to DRAM scratch.
  A:  attention per head in the S^T orientation (softmax over the PSUM
      partition axis via ones-matmul column sums), normalize after P@V.
  O:  head-sliced o_proj producing a partial [S, HID] output.
Host sums the 4 per-batch partials.

All matmuls run as float32r (FP22 multiply, fp32 accumulate) with moving
dim 512 for full PE rate.
"""
import numpy as np

import concourse.bass as bass
import concourse.tile as tile
from concourse import bacc
from concourse import mybir
from concourse.bass_utils import run_bass_kernel_spmd

# ---- problem dims (hardcoded) ----
B, S, HID = 2, 2048, 2048
NH = 16
QLORA, KVLORA = 1024, 512
NOPE, ROPE, VD = 128, 64, 128
QK = NOPE + ROPE
EPS = 1e-6
NHL = 4               # heads per core
P = 128
FD = 512              # matmul moving/free dim
SCALE = 1.0 / float(np.sqrt(np.float32(QK)))

F32 = mybir.dt.float32
F32R = mybir.dt.float32r
AF = mybir.ActivationFunctionType

NEWTON_RSQRT = True


def _r(ap):
    return ap.bitcast(F32R)


def build_nc(s=S):
    """Build the SPMD Bass program (same for all 8 cores)."""
    assert s % FD == 0
    sq = min(FD, s)        # s-quarter width for the projection phase
    n_sq = s // sq
    n_kc = s // P          # attention k chunks
    n_qt = s // FD         # attention q tiles
    nc = bacc.Bacc()

    xT = nc.dram_tensor("xT", [HID, s], F32, kind="ExternalInput")
    w_down = nc.dram_tensor("w_down", [HID, 1664], F32, kind="ExternalInput")
    w_qu = nc.dram_tensor("w_qu", [QLORA, 1024], F32, kind="ExternalInput")
    w_kvu_kn = nc.dram_tensor("w_kvu_kn", [KVLORA, 512], F32, kind="ExternalInput")
    w_kvu_v = nc.dram_tensor("w_kvu_v", [KVLORA, 512], F32, kind="ExternalInput")
    w_o = nc.dram_tensor("w_o", [512, HID], F32, kind="ExternalInput")
    cos2_d = nc.dram_tensor("cos2", [P, s], F32, kind="ExternalInput")
    sinp_d = nc.dram_tensor("sin_pre", [P, s], F32, kind="ExternalInput")
    mask_d = nc.dram_tensor("mask_p", [P, n_kc], F32, kind="ExternalInput")
    normw_d = nc.dram_tensor("normw", [1, 1536], F32, kind="ExternalInput")
    ones_d = nc.dram_tensor("ones_col", [P, 1], F32, kind="ExternalInput")

    out_d = nc.dram_tensor("out", [s, HID], F32, kind="ExternalOutput")

    QT_d = nc.dram_tensor("qt_scratch", [8, P, s], F32R)
    KT_d = nc.dram_tensor("kt_scratch", [5, P, s], F32R)
    V_d = nc.dram_tensor("v_scratch", [n_kc, P, 512], F32R)

    w_down3 = w_down[:, :].bitcast(F32R).rearrange("(hk p) c -> p hk c", p=P)
    xT3 = xT[:, :].bitcast(F32R).rearrange("(hk p) t -> p hk t", p=P)

    with tile.TileContext(nc) as tc:
      with tc.tile_pool(name="consts", bufs=1) as consts:
        mask_sb = consts.tile([P, n_kc], F32)
        nc.sync.dma_start(mask_sb, mask_d[:, :])
        normw = consts.tile([1, 1536], F32R)
        nc.sync.dma_start(normw, normw_d[:, :].bitcast(F32R))
        ones_col = consts.tile([P, 1], F32R)
        nc.sync.dma_start(ones_col, ones_d[:, :].bitcast(F32R))

        # ---------------- P: projections, per s-quarter ----------------
        with tc.tile_pool(name="pw", bufs=1) as pw, \
             tc.tile_pool(name="pwd", bufs=3) as pwd, \
             tc.tile_pool(name="px", bufs=1) as px, \
             tc.tile_pool(name="plat", bufs=1) as plat, \
             tc.tile_pool(name="ptmp", bufs=2) as ptmp, \
             tc.tile_pool(name="pout", bufs=3) as pout, \
             tc.tile_pool(name="pst", bufs=1) as pst, \
             tc.tile_pool(name="ps_mm", bufs=3, space="PSUM") as ps_mm, \
             tc.tile_pool(name="ps_sum", bufs=1, space="PSUM") as ps_sum, \
             tc.tile_pool(name="ps_scl", bufs=2, space="PSUM") as ps_scl:
            cos2 = pw.tile([P, s], F32)
            sinp = pw.tile([P, s], F32)
            def rope_from(src, out_t, tmp_t, tsw_t, sl):
                """out = x*cos2 + blockswap(x*sin_pre); x rows 64:128 zero.

                DVE ops can't read across partitions, so the 32-row block
                swap goes through two SBUF->SBUF DMAs into tsw_t.
                """
                nc.vector.tensor_mul(tmp_t, src, sinp[:, sl])
                nc.vector.tensor_mul(out_t, src, cos2[:, sl])
                nc.sync.dma_start(tsw_t[0:32], tmp_t[32:64])
                nc.sync.dma_start(tsw_t[32:64], tmp_t[0:32])
                nc.vector.tensor_add(out_t[0:64], out_t[0:64], tsw_t[0:64])

            wqu = pw.tile([P, 8, 1024], F32R)
            wkn = pw.tile([P, 4, 512], F32R)
            wv = pw.tile([P, 4, 512], F32R)

            w_qu3 = w_qu[:, :].bitcast(F32R).rearrange("(kk p) c -> p kk c", p=P)
            _pw_loads = [
                lambda: nc.sync.dma_start(cos2, cos2_d[:, :]),
                lambda: nc.sync.dma_start(sinp, sinp_d[:, :]),
                lambda: nc.sync.dma_start(
                    wkn, w_kvu_kn[:, :].bitcast(F32R).rearrange("(kk p) c -> p kk c", p=P)),
                lambda: nc.sync.dma_start(
                    wv, w_kvu_v[:, :].bitcast(F32R).rearrange("(kk p) c -> p kk c", p=P)),
                lambda: nc.sync.dma_start(wqu[:, :, 0:256], w_qu3[:, :, 0:256]),
                lambda: nc.sync.dma_start(wqu[:, :, 256:512], w_qu3[:, :, 256:512]),
                lambda: nc.sync.dma_start(wqu[:, :, 512:768], w_qu3[:, :, 512:768]),
                lambda: nc.sync.dma_start(wqu[:, :, 768:1024], w_qu3[:, :, 768:1024]),
            ]

            for q in range(n_sq):
                sl = slice(q * sq, (q + 1) * sq)
                xq = px.tile([P, 16, sq], F32R, tag="xq")
                nc.sync.dma_start(xq, xT3[:, :, sl])
                lat = [plat.tile([P, sq], F32R, tag=f"lat{rt}", name=f"lat{rt}")
                       for rt in range(12)]
                sumq_ps = ps_sum.tile([1, sq], F32, tag="sumq")
                sumkv_ps = ps_sum.tile([1, sq], F32, tag="sumkv")
                # -- down proj, stats, k rope --
                for rt in range(13):
                    wd_t = pwd.tile([P, 16, P], F32R, tag="wd")
                    nc.sync.dma_start(wd_t, w_down3[:, :, rt * P:(rt + 1) * P])
                    if q == 0 and 4 <= rt < 4 + len(_pw_loads):
                        _pw_loads[rt - 4]()
                    ps = ps_mm.tile([P, sq], F32, tag="dps")
                    for hk in range(16):
                        nc.tensor.matmul(
                            ps, _r(wd_t[:, hk, :]), _r(xq[:, hk, :]),
                            start=(hk == 0), stop=(hk == 15))
                    if rt < 12:
                        sq_t = ptmp.tile([P, sq], F32R, tag="sq")
                        nc.scalar.activation(sq_t, ps, AF.Square)
                        nc.vector.tensor_copy(lat[rt], ps)
                        tgt = sumq_ps if rt < 8 else sumkv_ps
                        nc.tensor.matmul(
                            tgt, _r(ones_col), _r(sq_t),
                            start=(rt in (0, 8)), stop=(rt in (7, 11)))
                    else:
                        tmp_t = ptmp.tile([P, sq], F32, tag="ropetmp")
                        tsw_t = ptmp.tile([64, sq], F32, tag="ropesw")
                        krt = pout.tile([P, sq], F32R, tag="krout")
                        rope_from(ps, krt, tmp_t, tsw_t, sl)
                        nc.sync.dma_start(KT_d[4, :, sl], krt)
                # -- rmsnorm scales: rsqrt(sum/n + eps), Newton-refined --
                rsqs = []
                for nm, sums, n_el in (("q", sumq_ps, QLORA), ("kv", sumkv_ps, KVLORA)):
                    m_t = pst.tile([1, sq], F32, tag=f"m{nm}", name=f"m{nm}")
                    nc.vector.tensor_scalar(
                        m_t, sums, 1.0 / n_el, EPS,
                        op0=mybir.AluOpType.mult, op1=mybir.AluOpType.add)
                    inv_t = pst.tile([1, sq], F32, tag=f"inv{nm}", name=f"inv{nm}")
                    nc.vector.reciprocal(inv_t, m_t)
                    y_t = pst.tile([1, sq], F32R, tag=f"y{nm}", name=f"y{nm}")
                    nc.scalar.activation(y_t, inv_t, AF.Sqrt)
                    if NEWTON_RSQRT:
                        t_t = pst.tile([1, sq], F32, tag=f"t{nm}", name=f"t{nm}")
                        nc.vector.tensor_mul(t_t, y_t, y_t)
                        nc.vector.tensor_mul(t_t, t_t, m_t)
                        nc.vector.tensor_scalar(
                            t_t, t_t, -0.5, 1.5,
                            op0=mybir.AluOpType.mult, op1=mybir.AluOpType.add)
                        nc.vector.tensor_mul(y_t, y_t, t_t)
                    rsqs.append(y_t)
                for rt in range(12):
                    scl = ps_scl.tile([P, sq], F32, tag="scl")
                    rs = rsqs[0] if rt < 8 else rsqs[1]
                    nc.tensor.matmul(
                        scl, _r(normw[:, rt * P:(rt + 1) * P]), _r(rs),
                        start=True, stop=True)
                    nc.vector.tensor_mul(lat[rt], lat[rt], scl)
                # -- up projections on in-SBUF latents --
                for ro in range(8):
                    ps = ps_mm.tile([P, sq], F32, tag="dps")
                    for kk in range(8):
                        nc.tensor.matmul(
                            ps, _r(wqu[:, kk, ro * P:(ro + 1) * P]), _r(lat[kk]),
                            start=(kk == 0), stop=(kk == 7))
                    ot = pout.tile([P, sq], F32R, tag="qo")
                    if ro < 4:
                        nc.any.tensor_copy(ot, ps)
                    else:
                        tmp_t = ptmp.tile([P, sq], F32, tag="ropetmp")
                        tsw_t = ptmp.tile([64, sq], F32, tag="ropesw")
                        rope_from(ps, ot, tmp_t, tsw_t, sl)
                    nc.sync.dma_start(QT_d[ro, :, sl], ot)
                for i in range(4):
                    ps = ps_mm.tile([P, sq], F32, tag="dps")
                    for kk in range(4):
                        nc.tensor.matmul(
                            ps, _r(wkn[:, kk, i * P:(i + 1) * P]), _r(lat[8 + kk]),
                            start=(kk == 0), stop=(kk == 3))
                    ot = pout.tile([P, sq], F32R, tag="qo")
                    nc.any.tensor_copy(ot, ps)
                    nc.sync.dma_start(KT_d[i, :, sl], ot)
                for sc in range(sq // P):
                    ps = ps_mm.tile([P, 512], F32, tag="dps")
                    for kk in range(4):
                        nc.tensor.matmul(
                            ps, _r(lat[8 + kk][:, sc * P:(sc + 1) * P]), _r(wv[:, kk, :]),
                            start=(kk == 0), stop=(kk == 3))
                    ot = pout.tile([P, 512], F32R, tag="vo")
                    nc.any.tensor_copy(ot, ps)
                    nc.sync.dma_start(V_d[q * (sq // P) + sc], ot)

        # ---------------- A: attention + O: o_proj ----------------
        with tc.tile_pool(name="aw", bufs=1) as aw:
            kr = aw.tile([P, s], F32R)
            vt = aw.tile([P, n_kc, 512], F32R)
            wo_sb = aw.tile([P, 4, HID], F32R)
            nc.sync.dma_start(wo_sb, w_o[:, :].bitcast(F32R).rearrange("(c p) n -> p c n", p=P))
            for c in range(n_kc):
                nc.sync.dma_start(vt[:, c, :], V_d[c])
            for qq in range(n_sq):
                qsl0 = slice(qq * sq, (qq + 1) * sq)
                nc.sync.dma_start(kr[:, qsl0], KT_d[4, :, qsl0])
            attnT = aw.tile([P, 4, s], F32R)
            with tc.tile_pool(name="aq", bufs=2) as aq, \
                 tc.tile_pool(name="aexp", bufs=4) as aexp, \
                 tc.tile_pool(name="asm", bufs=2) as asm, \
                 tc.tile_pool(name="adram", bufs=2, space="DRAM") as adram, \
                 tc.tile_pool(name="ps_s", bufs=3, space="PSUM") as ps_s, \
                 tc.tile_pool(name="ps_pv", bufs=2, space="PSUM") as ps_pv, \
                 tc.tile_pool(name="ps_ss", bufs=2, space="PSUM") as ps_ss:
                for h in range(4):
                    kn = aq.tile([P, s], F32R, tag="kn")
                    qn = aq.tile([P, s], F32R, tag="qn")
                    qr = aq.tile([P, s], F32R, tag="qr")
                    for qq in range(n_sq):
                        qsl0 = slice(qq * sq, (qq + 1) * sq)
                        nc.sync.dma_start(kn[:, qsl0], KT_d[h, :, qsl0])
                        nc.sync.dma_start(qn[:, qsl0], QT_d[h, :, qsl0])
                        nc.sync.dma_start(qr[:, qsl0], QT_d[4 + h, :, qsl0])
                    for qt in range(n_qt):
                        qsl = slice(qt * FD, (qt + 1) * FD)
                        pv_ps = ps_pv.tile([P, FD], F32, tag="pv")
                        ssum_ps = ps_ss.tile([1, FD], F32, tag="ss")
                        for kc in range(n_kc):
                            ks = slice(kc * P, (kc + 1) * P)
                            s_ps = ps_s.tile([P, FD], F32, tag="s")
                            nc.tensor.matmul(s_ps, _r(kn[:, ks]), _r(qn[:, qsl]),
                                             start=True, stop=False)
                            nc.tensor.matmul(s_ps, _r(kr[:, ks]), _r(qr[:, qsl]),
                                             start=False, stop=True)
                            e_t = aexp.tile([P, FD], F32R, tag="e")
                            nc.scalar.activation(e_t, s_ps, AF.Exp,
                                                 bias=mask_sb[:, kc:kc + 1], scale=SCALE)
                            nc.tensor.matmul(pv_ps, _r(vt[:, kc, h * P:(h + 1) * P]), _r(e_t),
                                             start=(kc == 0), stop=(kc == n_kc - 1))
                            nc.tensor.matmul(ssum_ps, _r(ones_col), _r(e_t),
                                             start=(kc == 0), stop=(kc == n_kc - 1))
                        rec = asm.tile([1, FD], F32, tag="rec")
                        nc.vector.reciprocal(rec, ssum_ps)
                        rec_dr = adram.tile([1, FD], F32, tag="recd")
                        nc.sync.dma_start(rec_dr, rec)
                        recB = asm.tile([P, FD], F32, tag="recB")
                        nc.gpsimd.dma_start(recB, rec_dr.to_broadcast([P, FD]))
                        nc.vector.tensor_mul(attnT[:, h, qsl], pv_ps, recB)
            # ---------------- O: o_proj ----------------
            with tc.tile_pool(name="ps_o", bufs=4, space="PSUM") as ps_o, \
                 tc.tile_pool(name="oo", bufs=3) as oo:
                for sc in range(s // P):
                    for nt in range(HID // FD):
                        ps = ps_o.tile([P, FD], F32, tag="o")
                        for hh in range(4):
                            nc.tensor.matmul(
                                ps, _r(attnT[:, hh, sc * P:(sc + 1) * P]),
                                _r(wo_sb[:, hh, nt * FD:(nt + 1) * FD]),
                                start=(hh == 0), stop=(hh == 3))
                        ot = oo.tile([P, FD], F32, tag="ot")
                        nc.any.tensor_copy(ot, ps)
                        nc.sync.dma_start(
                            out_d[sc * P:(sc + 1) * P, nt * FD:(nt + 1) * FD], ot)
    nc.compile()
    return nc


# ---------------- host-side packing ----------------

def _pack_core_inputs(inputs, c, s=S):
    b, hg = c // 4, c % 4
    heads = range(NHL * hg, NHL * hg + NHL)
    f32 = np.float32
    hidden = np.ascontiguousarray(np.asarray(inputs["hidden_states"], dtype=f32)[b, :s])
    mask = np.asarray(inputs["attention_mask"], dtype=f32)[b, 0, 0, :s]
    w_q_down = np.asarray(inputs["w_q_down"], dtype=f32)
    w_kv_down = np.asarray(inputs["w_kv_down"], dtype=f32)
    w_q_up = np.asarray(inputs["w_q_up"], dtype=f32)
    w_kv_up = np.asarray(inputs["w_kv_up"], dtype=f32)
    w_o = np.asarray(inputs["w_o"], dtype=f32)
    cos = np.asarray(inputs["cos"], dtype=f32)[:s]
    sin = np.asarray(inputs["sin"], dtype=f32)[:s]

    w_down = np.ascontiguousarray(np.concatenate(
        [w_q_down, w_kv_down, np.zeros((HID, 64), f32)], axis=1))
    cols = [w_q_up[:, h * QK: h * QK + NOPE] for h in heads]
    for h in heads:
        cols.append(np.concatenate(
            [w_q_up[:, h * QK + NOPE: (h + 1) * QK], np.zeros((QLORA, 64), f32)], axis=1))
    w_qu = np.ascontiguousarray(np.concatenate(cols, axis=1))
    w_kvu_kn = np.ascontiguousarray(np.concatenate(
        [w_kv_up[:, h * (NOPE + VD): h * (NOPE + VD) + NOPE] for h in heads], axis=1))
    w_kvu_v = np.ascontiguousarray(np.concatenate(
        [w_kv_up[:, h * (NOPE + VD) + NOPE: (h + 1) * (NOPE + VD)] for h in heads], axis=1))
    w_o_hg = np.ascontiguousarray(np.concatenate(
        [w_o[h * VD: (h + 1) * VD, :] for h in heads], axis=0))

    cosT = cos.T
    sinT = sin.T
    cos2 = np.ascontiguousarray(np.vstack([cosT, cosT]))
    sp64 = np.vstack([sinT[:32], -sinT[:32]])
    sin_pre = np.ascontiguousarray(np.vstack([sp64, sp64]))
    mask_p = np.ascontiguousarray(mask.reshape(s // P, P).T)
    normw = np.concatenate(
        [np.asarray(inputs["q_norm_w"], dtype=f32),
         np.asarray(inputs["kv_norm_w"], dtype=f32)]).reshape(1, 1536)

    return {
        "xT": np.ascontiguousarray(hidden.T),
        "w_down": w_down,
        "w_qu": w_qu,
        "w_kvu_kn": w_kvu_kn,
        "w_kvu_v": w_kvu_v,
        "w_o": w_o_hg,
        "cos2": cos2,
        "sin_pre": sin_pre,
        "mask_p": mask_p,
        "normw": np.ascontiguousarray(normw),
        "ones_col": np.ones((P, 1), f32),
    }


_NC_CACHE = {}


def kernel(**inputs) -> np.ndarray:
    if "nc" not in _NC_CACHE:
        _NC_CACHE["nc"] = build_nc()
    nc = _NC_CACHE["nc"]
    in_maps = [_pack_core_inputs(inputs, c) for c in range(8)]
    res = run_bass_kernel_spmd(nc, in_maps, core_ids=list(range(8)))
    outs = [res.results[c]["out"] for c in range(8)]
    full = np.stack([outs[0] + outs[1] + outs[2] + outs[3],
                     outs[4] + outs[5] + outs[6] + outs[7]]).astype(np.float32)
    return full

